# revision 24
# baseline (speedup 1.0000x reference)
"""Trainium2 Bass kernel for the quantized Conv2d (nn_Conv2d_47356309405843).

Reference semantics: x_q = fp8e5m2(x), w_q = fp8e5m2(w), then 72 masked
sub-convs (8 channel groups x 9 taps) with fp16 requantization of the
partial sum after every step.

This kernel drops the INTERMEDIATE fp16 requantization and accumulates
the whole conv in PSUM fp32 (final result rounded once to fp16). On the
reference input distribution this changes the output by ~1.3e-3 relL2
vs the jax reference (vs 1.1e-3 for the bit-exact 72-step emulation) --
far inside the 2e-2 gate.

Shipped mode "dq3" (per core, batch-sharded 2 images/core over 8 cores):
  - compute in the PADDED flat layout (58x58 per image), so every conv
    tap (ih, iw) is a pure column offset (58*ih + iw) into ONE flat
    plane; wrap columns/rows are computed but discarded on host.
  - SBUF holds 4 row-shifted copies of the fp8 plane (shift = 0/1/2/3
    rows) at [partition group 0/1] x [DoubleRow plane 0/1]. One fp8
    DoubleRow matmul with base column offset c then contracts taps
    (0,c),(1,c),(2,c) over all 64 c_in at once (256 rows/pass, the
    4th slot zero-weighted) -> 3 matmuls per 512-wide PSUM chunk,
    14 chunks per rep = 42 matmuls streaming ~10.8k PE cycles.
  - input DMA is split into column blocks on both HWDGE rings so
    chunk-0 matmuls start after the first block; a couple of warmup
    matmuls ramp the PE clock while x streams in.
  - drain PSUM f32 -> SBUF f16 alternating ScalarE/VectorE; output DMA
    in column groups (ring-alternated) so the store overlaps compute.
  - host strips the pad columns and upcasts f16 -> f32.

Older modes kept for benching: "pair5" (9 pre-shifted tap planes, 5
plain fp8 matmuls per chunk) and "dr3" (DoubleRow over pre-shifted
planes). dq3 is ~3x faster than pair5: 3.4x fewer PE row-cycles and
2.3x less input HBM traffic.
"""

import numpy as np
import ml_dtypes
from contextlib import ExitStack

import concourse.bass as bass
import concourse.tile as tile
from concourse import bacc, mybir
from concourse.bass_utils import run_bass_kernel_spmd

# problem constants (hardcoded per contract)
B, C_IN, H, W = 16, 64, 56, 56
C_OUT, K, PAD = 128, 3, 1
N_CORES = 8
B_PC = B // N_CORES                  # images per core
SPI = H * W                          # spatial per image 3136
NSTEP = (C_IN // 8) * K * K          # 72 reference accumulation steps

# output chunking: the tap shifts are pre-baked into the SBUF planes, so
# the matmul is a pure 1x1 conv over the flat per-core spatial dim
# (B_PC*SPI = 6272). Chunk it into full PSUM banks (512 fp32).
FREE = B_PC * SPI
CHUNKS = [(c0, min(512, FREE - c0)) for c0 in range(0, FREE, 512)]

# dq3 mode: compute in the PADDED flat layout (Hp x Wp = 58 x 58 per
# image) so all 9 taps of the 3x3 conv are plain column offsets of ONE
# plane. Four row-shifted copies of the padded plane (shifts 0/1/2/3
# rows) live at [partition group 0/1] x [DoubleRow plane 0/1]; a single
# DoubleRow matmul with base column offset c then contracts taps
# (0,c),(1,c),(2,c),(zero) over all 64 channels at once -> 3 matmuls
# per output chunk. Output is padded (wrap columns/rows discarded on
# host).
HP, WP = H + 2 * PAD, W + 2 * PAD
FREE_P = B_PC * HP * WP                      # 6728
LQ = FREE_P + 8                              # per-copy buffer (read margin)
CHUNKS_P = [(c0, min(512, FREE_P - c0)) for c0 in range(0, FREE_P, 512)]

# matmul packing mode: "pair5" = 5 normal K=128 fp8 matmuls per tile
# (2 taps x 64ch each); "dr3" = 3 DoubleRow matmuls (4 taps x 64ch each);
# "dq3" = padded-plane layout, 3 DoubleRow matmuls per chunk over 4
# row-shifted copies (no per-tap input replication)
MODE = "dq3"
# emission order: False = chunk-major; True = weight-stationary over
# 8-bank groups; "g4" = weight-stationary over 4-bank tiles, fused
# drains; "ws" = weight-stationary sweeps over groups [8,5,1] with
# per-chunk drains and a short tail
KORDER = "ws"
# default output-DMA grouping for dq3 (0 = single DMA at end of rep;
# nonzero = ring-alternated group DMAs as drains complete)
DMA_PG = 1

_TAPS = [(ih, iw) for ih in range(K) for iw in range(K)]


def _mm_descs(mode):
    """Per-matmul contraction layout: list of (Ki, planes, taps) where
    taps[plane][part_group] gives the (ih, iw) pre-shift of the 64-channel
    group at partitions [g*64:(g+1)*64], plane p. None = zero weights."""
    if mode == "pair5":
        # one plane, two 64-partition groups per matmul
        pairs = [((0, 0), (0, 1)), ((0, 2), (1, 0)), ((1, 1), (1, 2)),
                 ((2, 0), (2, 1)), ((2, 2), None)]
        return [dict(ki=128, planes=1, taps=[list(p)]) for p in pairs]
    elif mode == "dr3":
        return [
            dict(ki=128, planes=2, taps=[[(0, 0), (0, 1)], [(0, 2), (1, 0)]]),
            dict(ki=128, planes=2, taps=[[(1, 1), (1, 2)], [(2, 0), (2, 1)]]),
            # tap (2,2): 64 channels split across 2 planes of 32 partitions
            dict(ki=32, planes=2, taps=[[(2, 2)], [(2, 2)]], ch_split=True),
        ]
    raise ValueError(mode)


_COMPILED = {}


def _build(repeats=1, has_bias=False, mode=MODE, skip_drain=False,
           skip_dma=False, drain_eng="alt", korder=False, bench_opts=None):
    nc = bacc.Bacc("TRN2", target_bir_lowering=False, debug=False,
                   num_devices=N_CORES)
    if mode == "dq3":
        xin = nc.dram_tensor("xin", [128, 2 * LQ], mybir.dt.float8e5,
                             kind="ExternalInput").ap()
        win = nc.dram_tensor("win", [128, 3 * 2 * C_OUT], mybir.dt.float8e5,
                             kind="ExternalInput").ap()
        bin_ = (nc.dram_tensor("bin", [C_OUT, 1], mybir.dt.float32,
                               kind="ExternalInput").ap() if has_bias else None)
        yout = nc.dram_tensor("yout", [C_OUT, FREE_P], mybir.dt.float16,
                              kind="ExternalOutput").ap()
        with tile.TileContext(nc) as tc:
            with ExitStack() as ctx:
                _emit_dq3(tc, ctx, xin, win, yout, bin_, repeats,
                          skip_drain, skip_dma, drain_eng, korder,
                          bench_opts)
        nc.compile()
        return nc

    descs = _mm_descs(mode)
    xins, wins = [], []
    for k, d in enumerate(descs):
        ki, pl = d["ki"], d["planes"]
        xins.append(nc.dram_tensor(f"x{k}", [ki, pl * B_PC * H * W],
                                   mybir.dt.float8e5, kind="ExternalInput").ap())
        wins.append(nc.dram_tensor(f"w{k}", [ki, pl * C_OUT],
                                   mybir.dt.float8e5, kind="ExternalInput").ap())
    bin_ = (nc.dram_tensor("bin", [C_OUT, 1], mybir.dt.float32,
                           kind="ExternalInput").ap() if has_bias else None)
    yout = nc.dram_tensor("yout", [C_OUT, B_PC * SPI], mybir.dt.float16,
                          kind="ExternalOutput").ap()

    with tile.TileContext(nc) as tc:
        with ExitStack() as ctx:
            _emit(tc, ctx, descs, xins, wins, yout, bin_, repeats, mode,
                  skip_drain, skip_dma, drain_eng, korder, bench_opts)
    nc.compile()
    return nc


def _emit_dq3(tc, ctx, xin, win, yout, bin_, repeats, skip_drain=False,
              skip_dma=False, drain_eng="alt", korder=False, bench_opts=None):
    bench_opts = bench_opts or {}
    nc = tc.nc
    f8, f16, f32 = mybir.dt.float8e5, mybir.dt.float16, mybir.dt.float32
    dr = mybir.MatmulPerfMode.DoubleRow

    singles = ctx.enter_context(tc.tile_pool(name="singles", bufs=1))
    psum_pool = ctx.enter_context(tc.tile_pool(name="ps", bufs=8, space="PSUM"))
    out_pool = ctx.enter_context(tc.tile_pool(name="outs", bufs=3))

    in_blocks = bench_opts.get("in_blocks", 4)
    warm = bench_opts.get("warm", 2)
    indma_per_rep = bench_opts.get("indma_per_rep", 0)
    rings = [nc.sync, nc.scalar]  # the two HWDGE rings (SP / Activation)

    xin3 = xin.rearrange("c (p s) -> c p s", p=2)
    win4 = win.rearrange("c (m p o) -> c m p o", m=3, p=2)

    if indma_per_rep:
        in_pool = ctx.enter_context(tc.tile_pool(name="ins", bufs=2))
        xq = wt = None
    else:
        xq = singles.tile([128, 2, LQ], f8, name="xq")
        wt = singles.tile([128, 3, 2, C_OUT], f8, name="wt")

    def load_inputs(xq, wt):
        # weights first (small) on SP so warmup matmuls can start early;
        # x in column blocks alternating rings so chunk-0 matmuls start
        # after the first block rather than the whole 1.7MB
        nc.sync.dma_start(wt[:], win4)
        bounds = np.linspace(0, LQ, in_blocks + 1).astype(int)
        for i in range(in_blocks):
            j0, j1 = int(bounds[i]), int(bounds[i + 1])
            rings[i % 2].dma_start(xq[:, :, j0:j1], xin3[:, :, j0:j1])

    if not indma_per_rep:
        load_inputs(xq, wt)

    bias_sb = None
    if bin_ is not None:
        bias_sb = singles.tile([C_OUT, 1], f32)
        nc.sync.dma_start(bias_sb[:], bin_[:])

    nmm = min(3, bench_opts.get("nmm_cap") or 3)
    same_w = bench_opts.get("same_w", False)

    def one_mm(pt, c, c0, fw, start, stop):
        wc = 0 if same_w else c
        nc.tensor.matmul(pt[:, :fw], wt[0:128, wc, :, :],
                         xq[0:128, :, c0 + c:c0 + c + fw],
                         start=start, stop=stop, perf_mode=dr,
                         skip_group_check=bool(korder))

    def drain_slice(t, ysl, psl):
        if bias_sb is not None:
            nc.vector.tensor_scalar_add(ysl, psl, bias_sb[:, 0:1])
        elif drain_eng == "act" or (drain_eng == "alt" and t % 2 == 0):
            nc.scalar.copy(ysl, psl)
        else:
            nc.vector.tensor_copy(ysl, psl)

    dma_pg = bench_opts.get("dma_pg", DMA_PG)

    def emit_warm(xq_, wt_):
        # ramp the PE clock while the x blocks stream in: harmless f8
        # matmuls on the (small, already-resident) weight tile. PSUM comes
        # from the main pool tags so the 8-bank budget is respected.
        rhs = wt_[:].rearrange("k m p o -> k (m p o)")[:, 0:512]
        for wi in range(warm):
            if korder == "g4":
                pw = psum_pool.tile([C_OUT, 4, 512], f32, tag="ps4",
                                    bufs=2, name=f"warm{wi}")[:, 0, :]
            else:
                pw = psum_pool.tile([C_OUT, 512], f32, tag="ps", name=f"warm{wi}")
            nc.tensor.matmul(pw, wt_[0:128, 0, 0, :], rhs,
                             start=True, stop=True, skip_group_check=True)

    if not indma_per_rep and warm:
        emit_warm(xq, wt)

    for _rep in range(repeats):
        if indma_per_rep:
            xq = in_pool.tile([128, 2, LQ], f8, tag="xq")
            wt = in_pool.tile([128, 3, 2, C_OUT], f8, tag="wt")
            load_inputs(xq, wt)
            if warm:
                emit_warm(xq, wt)
        y16 = out_pool.tile([C_OUT, FREE_P], f16, tag="y16")
        if korder == "g4":
            for gi, g0 in enumerate(range(0, len(CHUNKS_P), 4)):
                grp = CHUNKS_P[g0:g0 + 4]
                pt = psum_pool.tile([C_OUT, 4, 512], f32, tag="ps4",
                                    bufs=2, name=f"pt{gi}")
                for c in range(nmm):
                    for b, (c0, fw) in enumerate(grp):
                        one_mm(pt[:, b, :], c, c0, fw, c == 0, c == nmm - 1)
                if skip_drain:
                    continue
                gc0, gcw = grp[0][0], sum(fw for _, fw in grp)
                psl = pt[:, 0:len(grp), :].rearrange("o b f -> o (b f)")[:, :gcw]
                drain_slice(gi, y16[:, gc0:gc0 + gcw], psl)
                if dma_pg and not skip_dma:
                    rings[gi % 2].dma_start(yout[:, gc0:gc0 + gcw],
                                            y16[:, gc0:gc0 + gcw])
        elif korder == "ws":
            # weight-stationary sweeps over chunk groups (3 LD_WEIGHTS per
            # group); small last group so the rep tail is one drain + one
            # short DMA
            gb = [0, 8, 13, 14]
            for gi in range(len(gb) - 1):
                grp = CHUNKS_P[gb[gi]:gb[gi + 1]]
                pts = [psum_pool.tile([C_OUT, 512], f32, tag="ps",
                                      name=f"pt{gb[gi] + i}")
                       for i in range(len(grp))]
                for c in range(nmm):
                    for pt, (c0, fw) in zip(pts, grp):
                        one_mm(pt, c, c0, fw, c == 0, c == nmm - 1)
                if skip_drain:
                    continue
                for t, (pt, (c0, fw)) in enumerate(zip(pts, grp)):
                    drain_slice(gb[gi] + t, y16[:, c0:c0 + fw], pt[:, :fw])
                if dma_pg and not skip_dma:
                    gc0 = grp[0][0]
                    gc1 = grp[-1][0] + grp[-1][1]
                    rings[gi % 2].dma_start(yout[:, gc0:gc1],
                                            y16[:, gc0:gc1])
        elif korder:
            # weight-stationary over 8-bank groups: 3 LD_WEIGHTS per group
            for g0 in range(0, len(CHUNKS_P), 8):
                grp = CHUNKS_P[g0:g0 + 8]
                pts = [psum_pool.tile([C_OUT, 512], f32, tag="ps",
                                      name=f"pt{g0 + i}")
                       for i in range(len(grp))]
                for c in range(nmm):
                    for pt, (c0, fw) in zip(pts, grp):
                        one_mm(pt, c, c0, fw, c == 0, c == nmm - 1)
                if not skip_drain:
                    for t, (pt, (c0, fw)) in enumerate(zip(pts, grp)):
                        drain_slice(g0 + t, y16[:, c0:c0 + fw], pt[:, :fw])
        else:
            # with dma_pg: issue the output DMA in groups as drains complete
            # (ring-alternated), keeping the LAST group small so the
            # post-compute tail is one drain + one short DMA
            nt = len(CHUNKS_P)
            bounds = ([] if not dma_pg else
                      [nt * (i + 1) // max(dma_pg - 1, 1)
                       for i in range(max(dma_pg - 1, 1))][:-1] + [nt - 1, nt])
            prev = 0
            for t, (c0, fw) in enumerate(CHUNKS_P):
                pt = psum_pool.tile([C_OUT, 512], f32, tag="ps")
                for c in range(nmm):
                    one_mm(pt, c, c0, fw, c == 0, c == nmm - 1)
                if not skip_drain:
                    drain_slice(t, y16[:, c0:c0 + fw], pt[:, :fw])
                    if dma_pg and not skip_dma and t + 1 in bounds:
                        gc0 = CHUNKS_P[prev][0]
                        gc1 = c0 + fw
                        rings[len([b for b in bounds if b <= t]) % 2].dma_start(
                            yout[:, gc0:gc1], y16[:, gc0:gc1])
                        prev = t + 1
        if not (skip_drain or skip_dma or
                (dma_pg and korder in (False, "g4", "ws"))):
            nc.sync.dma_start(yout[:, :], y16[:])


def _emit(tc, ctx, descs, xins, wins, yout, bin_, repeats, mode,
          skip_drain=False, skip_dma=False, drain_eng="alt", korder=False,
          bench_opts=None):
    bench_opts = bench_opts or {}
    nc = tc.nc
    f8, f16, f32 = mybir.dt.float8e5, mybir.dt.float16, mybir.dt.float32
    dr = (mybir.MatmulPerfMode.DoubleRow if mode == "dr3" else None)

    singles = ctx.enter_context(tc.tile_pool(name="singles", bufs=1))
    psum_pool = ctx.enter_context(tc.tile_pool(name="ps", bufs=8, space="PSUM"))
    out_pool = ctx.enter_context(tc.tile_pool(name="outs", bufs=3))

    xgs, wts = [], []
    for k, d in enumerate(descs):
        ki, pl = d["ki"], d["planes"]
        xg = singles.tile([ki, pl, FREE], f8, name=f"xg{k}")
        wt = singles.tile([ki, pl, C_OUT], f8, name=f"wt{k}")
        nc.sync.dma_start(xg[0:ki], xins[k].rearrange(
            "c (p s) -> c p s", p=pl))
        nc.sync.dma_start(wt[0:ki], wins[k].rearrange(
            "c (p o) -> c p o", p=pl))
        xgs.append(xg)
        wts.append(wt)

    bias_sb = None
    if bin_ is not None:
        bias_sb = singles.tile([C_OUT, 1], f32)
        nc.sync.dma_start(bias_sb[:], bin_[:])

    nmm = len(descs)

    same_w = bench_opts.get("same_w", False)
    nmm_cap = bench_opts.get("nmm_cap")

    def one_mm(pt, k, c0, fw, start, stop):
        if same_w:
            k = 0
        d = descs[k]
        ki = d["ki"]
        if dr is not None:
            lhsT = wts[k][0:ki, :, :]
            rhs = xgs[k][0:ki, :, c0:c0 + fw]
        else:
            lhsT = wts[k][0:ki, 0, :]
            rhs = xgs[k][0:ki, 0, c0:c0 + fw]
        nc.tensor.matmul(pt[:, :fw], lhsT, rhs, start=start, stop=stop,
                         perf_mode=dr, skip_group_check=bool(korder))

    def drain(t, pt, c0, fw, y16):
        ysl = y16[:, c0:c0 + fw]
        if bias_sb is not None:
            nc.vector.tensor_scalar_add(ysl, pt[:, :fw], bias_sb[:, 0:1])
        elif drain_eng == "act" or (drain_eng == "alt" and t % 2 == 0):
            nc.scalar.copy(ysl, pt[:, :fw])
        else:
            nc.vector.tensor_copy(ysl, pt[:, :fw])

    for _rep in range(repeats):
        y16 = out_pool.tile([C_OUT, FREE], f16, tag="y16")
        if korder == "g4":
            # weight-stationary over 4-bank PSUM tiles; one fused drain
            # per group (4 banks contiguous in PSUM -> single ACT/DVE op)
            for gi, g0 in enumerate(range(0, len(CHUNKS), 4)):
                grp = CHUNKS[g0:g0 + 4]
                pt = psum_pool.tile([C_OUT, 4, 512], f32, tag="ps4",
                                    bufs=2, name=f"pt{gi}")
                for k in range(nmm):
                    for b, (c0, fw) in enumerate(grp):
                        one_mm(pt[:, b, :], k, c0, fw, k == 0, k == nmm - 1)
                if skip_drain:
                    continue
                gc0, gcw = grp[0][0], sum(fw for _, fw in grp)
                ysl = y16[:, gc0:gc0 + gcw]
                psl = pt[:, 0:len(grp), :].rearrange("o b f -> o (b f)") \
                    if gcw == 512 * len(grp) else pt[:, 0, :gcw]
                if bias_sb is not None:
                    nc.vector.tensor_scalar_add(ysl, psl, bias_sb[:, 0:1])
                elif drain_eng == "act" or (drain_eng == "alt" and gi % 2 == 0):
                    nc.scalar.copy(ysl, psl)
                else:
                    nc.vector.tensor_copy(ysl, psl)
        elif korder:
            # weight-stationary: for each PSUM-bank group, sweep each
            # stationary over all banks before switching weights
            for g0 in range(0, len(CHUNKS), 8):
                grp = CHUNKS[g0:g0 + 8]
                pts = [psum_pool.tile([C_OUT, 512], f32, tag="ps",
                                      name=f"pt{g0 + i}")
                       for i in range(len(grp))]
                for k in range(nmm):
                    for pt, (c0, fw) in zip(pts, grp):
                        one_mm(pt, k, c0, fw, k == 0, k == nmm - 1)
                if not skip_drain:
                    for t, (pt, (c0, fw)) in enumerate(zip(pts, grp)):
                        drain(g0 + t, pt, c0, fw, y16)
        else:
            for t, (c0, fw) in enumerate(CHUNKS):
                pt = psum_pool.tile([C_OUT, 512], f32, tag="ps")
                n_emit = min(nmm, nmm_cap) if nmm_cap else nmm
                for k in range(n_emit):
                    one_mm(pt, k, c0, fw, k == 0, k == n_emit - 1)
                if not skip_drain:
                    drain(t, pt, c0, fw, y16)
        if not (skip_drain or skip_dma):
            nc.sync.dma_start(yout[:, :], y16[:])


def _prep_inputs_dq3(x, weight):
    """dq3 layout: per core, 4 row-shifted copies of the padded flat plane
    at [partition group 0/1] x [plane 0/1]; shift r = g + 2p rows."""
    f8 = ml_dtypes.float8_e5m2
    xq = x.astype(f8)
    wq = weight.astype(f8)                       # [C_OUT, C_IN, K, K]

    wb = np.zeros((128, 3, 2, C_OUT), f8)
    for c in range(K):
        for g in range(2):
            for p in range(2):
                r = g + 2 * p
                if r < K:
                    wb[64 * g:64 * (g + 1), c, p, :] = wq[:, :, r, c].T
    wflat = np.ascontiguousarray(wb.reshape(128, 3 * 2 * C_OUT))

    in_maps = []
    for core in range(N_CORES):
        xs = xq[core * B_PC:(core + 1) * B_PC]   # [B_PC, C_IN, H, W]
        xp = np.zeros((B_PC, C_IN, HP, WP), f8)
        xp[:, :, PAD:PAD + H, PAD:PAD + W] = xs
        flat = xp.transpose(1, 0, 2, 3).reshape(C_IN, FREE_P)
        buf = np.zeros((128, 2, LQ), f8)
        for g in range(2):
            for p in range(2):
                r = g + 2 * p
                n = FREE_P - WP * r
                buf[64 * g:64 * (g + 1), p, :n] = flat[:, WP * r:]
        in_maps.append({"xin": np.ascontiguousarray(buf.reshape(128, 2 * LQ)),
                        "win": wflat})
    return in_maps


def _prep_inputs(x, weight, mode=MODE):
    """Host-side quantize + tap-pre-shifted layout. Per-core input maps."""
    if mode == "dq3":
        return _prep_inputs_dq3(x, weight)
    f8 = ml_dtypes.float8_e5m2
    descs = _mm_descs(mode)
    xq = x.astype(f8)
    wq = weight.astype(f8)                       # [C_OUT, C_IN, K, K]

    # weights: per mm, [Ki, planes*C_OUT]
    wbufs = []
    for d in descs:
        ki, pl = d["ki"], d["planes"]
        wb = np.zeros((ki, pl, C_OUT), f8)
        for p in range(pl):
            for g, tap in enumerate(d["taps"][p]):
                if tap is None:
                    continue
                ih, iw = tap
                if d.get("ch_split"):
                    cs = slice(p * ki, (p + 1) * ki)   # plane selects channels
                    wb[:, p, :] = wq[:, cs, ih, iw].T
                else:
                    wb[g * 64:(g + 1) * 64, p, :] = wq[:, :, ih, iw].T
        wbufs.append(np.ascontiguousarray(wb.reshape(ki, pl * C_OUT)))

    in_maps = []
    for core in range(N_CORES):
        xs = xq[core * B_PC:(core + 1) * B_PC]   # [B_PC, C_IN, H, W]
        xp = np.zeros((B_PC, C_IN, H + 2 * PAD, W + 2 * PAD), f8)
        xp[:, :, PAD:PAD + H, PAD:PAD + W] = xs
        m = {}
        for k, d in enumerate(descs):
            ki, pl = d["ki"], d["planes"]
            xb = np.zeros((ki, pl, B_PC, H, W), f8)
            for p in range(pl):
                for g, tap in enumerate(d["taps"][p]):
                    if tap is None:
                        continue
                    ih, iw = tap
                    win = xp[:, :, ih:ih + H, iw:iw + W]   # [B_PC, C_IN, H, W]
                    if d.get("ch_split"):
                        xb[:, p] = win[:, p * ki:(p + 1) * ki].transpose(1, 0, 2, 3)
                    else:
                        xb[g * 64:(g + 1) * 64, p] = win.transpose(1, 0, 2, 3)
            m[f"x{k}"] = np.ascontiguousarray(xb.reshape(ki, pl * B_PC * H * W))
            m[f"w{k}"] = wbufs[k]
        in_maps.append(m)
    return in_maps


def kernel(x, weight, bias, _trace=False):
    x = np.asarray(x, np.float32)
    weight = np.asarray(weight, np.float32)
    bias = np.asarray(bias, np.float32)
    has_bias = bool(np.any(bias))

    key = (MODE, KORDER, has_bias)
    if key not in _COMPILED:
        _COMPILED[key] = _build(has_bias=has_bias, korder=KORDER)
    nc = _COMPILED[key]

    in_maps = _prep_inputs(x, weight)
    if has_bias:
        # reference adds bias once per accumulation step (72 times total)
        beff = (NSTEP * bias).reshape(C_OUT, 1).astype(np.float32)
        for m in in_maps:
            m["bin"] = np.ascontiguousarray(beff)
    res = run_bass_kernel_spmd(nc, in_maps, list(range(N_CORES)),
                               trace=_trace)

    y = np.empty((B, C_OUT, H, W), np.float32)
    for core in range(N_CORES):
        yo = res.results[core]["yout"]
        if MODE == "dq3":
            yo = yo.reshape(C_OUT, B_PC, HP, WP)[:, :, :H, :W]
        else:
            yo = yo.reshape(C_OUT, B_PC, H, W)
        y[core * B_PC:(core + 1) * B_PC] = \
            yo.astype(np.float32).transpose(1, 0, 2, 3)
    if _trace:
        return y, res
    return y



# revision 33
# speedup vs baseline: 2.1274x; 2.1274x over previous
"""Trainium2 Bass kernel for the quantized Conv2d (nn_Conv2d_47356309405843).

Reference semantics: x_q = fp8e5m2(x), w_q = fp8e5m2(w), then 72 masked
sub-convs (8 channel groups x 9 taps) with fp16 requantization of the
partial sum after every step.

This kernel drops the INTERMEDIATE fp16 requantization and accumulates
the whole conv in PSUM fp32 (final result rounded once to fp16). On the
reference input distribution this changes the output by ~1.3e-3 relL2
vs the jax reference (vs 1.1e-3 for the bit-exact 72-step emulation) --
far inside the 2e-2 gate.

Shipped mode "dq3" (per core, batch-sharded 2 images/core over 8 cores):
  - compute in the PADDED flat layout (58x58 per image), so every conv
    tap (ih, iw) is a pure column offset (58*ih + iw) into ONE flat
    plane; wrap columns/rows are computed but discarded on host.
  - SBUF holds 4 row-shifted copies of the fp8 plane (shift = 0/1/2/3
    rows) at [partition group 0/1] x [DoubleRow plane 0/1]. One fp8
    DoubleRow matmul with base column offset c then contracts taps
    (0,c),(1,c),(2,c) over all 64 c_in at once (256 rows/pass, the
    4th slot zero-weighted) -> 3 matmuls per 512-wide PSUM chunk. Only
    the 56 valid output rows per image are computed (pad rows skipped):
    14 chunks per rep = 42 matmuls streaming ~9.7k PE cycles.
  - input DMA is split into column blocks on both HWDGE rings so
    chunk-0 matmuls start after the first block; a couple of warmup
    matmuls ramp the PE clock while x streams in.
  - drain PSUM f32 -> SBUF f16 alternating ScalarE/VectorE; output DMA
    in column groups (ring-alternated) so the store overlaps compute.
  - host strips the pad columns and upcasts f16 -> f32.

Older modes kept for benching: "pair5" (9 pre-shifted tap planes, 5
plain fp8 matmuls per chunk) and "dr3" (DoubleRow over pre-shifted
planes). dq3 is ~3x faster than pair5: 3.4x fewer PE row-cycles and
2.3x less input HBM traffic.
"""

import numpy as np
import ml_dtypes
from contextlib import ExitStack

import concourse.bass as bass
import concourse.tile as tile
from concourse import bacc, mybir
from concourse.bass_utils import run_bass_kernel_spmd

# problem constants (hardcoded per contract)
B, C_IN, H, W = 16, 64, 56, 56
C_OUT, K, PAD = 128, 3, 1
N_CORES = 8
B_PC = B // N_CORES                  # images per core
SPI = H * W                          # spatial per image 3136
NSTEP = (C_IN // 8) * K * K          # 72 reference accumulation steps

# output chunking: the tap shifts are pre-baked into the SBUF planes, so
# the matmul is a pure 1x1 conv over the flat per-core spatial dim
# (B_PC*SPI = 6272). Chunk it into full PSUM banks (512 fp32).
FREE = B_PC * SPI
CHUNKS = [(c0, min(512, FREE - c0)) for c0 in range(0, FREE, 512)]

# dq3 mode: compute in the PADDED flat layout (Hp x Wp = 58 x 58 per
# image) so all 9 taps of the 3x3 conv are plain column offsets of ONE
# plane. Four row-shifted copies of the padded plane (shifts 0/1/2/3
# rows) live at [partition group 0/1] x [DoubleRow plane 0/1]; a single
# DoubleRow matmul with base column offset c then contracts taps
# (0,c),(1,c),(2,c),(zero) over all 64 channels at once -> 3 matmuls
# per output chunk. Output is padded (wrap columns/rows discarded on
# host).
HP, WP = H + 2 * PAD, W + 2 * PAD
FREE_P = B_PC * HP * WP                      # 6728 (full padded plane)
LQ = FREE_P + 8                              # per-copy buffer (read margin)
# only the H=56 valid output rows per image are computed/drained/stored;
# the 2 pad rows per image are skipped (pad COLUMNS stay interleaved and
# are stripped on host). chunk = (src col in padded plane, dst col in the
# dense output, width)
FREE_V = B_PC * H * WP                       # 6496
CHUNKS_P = []
for _img in range(B_PC):
    for _off in range(0, H * WP, 512):
        CHUNKS_P.append((_img * HP * WP + _off, _img * H * WP + _off,
                         min(512, H * WP - _off)))

# matmul packing mode: "pair5" = 5 normal K=128 fp8 matmuls per tile
# (2 taps x 64ch each); "dr3" = 3 DoubleRow matmuls (4 taps x 64ch each);
# "dq3" = padded-plane layout, 3 DoubleRow matmuls per chunk over 4
# row-shifted copies (no per-tap input replication)
MODE = "dq3"
# emission order: False = chunk-major; True = weight-stationary over
# 8-bank groups; "g4" = weight-stationary over 4-bank tiles, fused
# drains; "ws" = weight-stationary sweeps over groups [8,5,1] with
# per-chunk drains and a short tail
KORDER = "ws"
# default output-DMA grouping for dq3 (0 = single DMA at end of rep;
# nonzero = ring-alternated group DMAs as drains complete)
DMA_PG = 1

_TAPS = [(ih, iw) for ih in range(K) for iw in range(K)]


def _mm_descs(mode):
    """Per-matmul contraction layout: list of (Ki, planes, taps) where
    taps[plane][part_group] gives the (ih, iw) pre-shift of the 64-channel
    group at partitions [g*64:(g+1)*64], plane p. None = zero weights."""
    if mode == "pair5":
        # one plane, two 64-partition groups per matmul
        pairs = [((0, 0), (0, 1)), ((0, 2), (1, 0)), ((1, 1), (1, 2)),
                 ((2, 0), (2, 1)), ((2, 2), None)]
        return [dict(ki=128, planes=1, taps=[list(p)]) for p in pairs]
    elif mode == "dr3":
        return [
            dict(ki=128, planes=2, taps=[[(0, 0), (0, 1)], [(0, 2), (1, 0)]]),
            dict(ki=128, planes=2, taps=[[(1, 1), (1, 2)], [(2, 0), (2, 1)]]),
            # tap (2,2): 64 channels split across 2 planes of 32 partitions
            dict(ki=32, planes=2, taps=[[(2, 2)], [(2, 2)]], ch_split=True),
        ]
    raise ValueError(mode)


_COMPILED = {}


def _build(repeats=1, has_bias=False, mode=MODE, skip_drain=False,
           skip_dma=False, drain_eng="alt", korder=False, bench_opts=None):
    nc = bacc.Bacc("TRN2", target_bir_lowering=False, debug=False,
                   num_devices=N_CORES)
    if mode == "dq3":
        xin = nc.dram_tensor("xin", [128, 2 * LQ], mybir.dt.float8e5,
                             kind="ExternalInput").ap()
        win = nc.dram_tensor("win", [128, 3 * 2 * C_OUT], mybir.dt.float8e5,
                             kind="ExternalInput").ap()
        bin_ = (nc.dram_tensor("bin", [C_OUT, 1], mybir.dt.float32,
                               kind="ExternalInput").ap() if has_bias else None)
        yout = nc.dram_tensor("yout", [C_OUT, FREE_V], mybir.dt.float16,
                              kind="ExternalOutput").ap()
        with tile.TileContext(nc) as tc:
            with ExitStack() as ctx:
                _emit_dq3(tc, ctx, xin, win, yout, bin_, repeats,
                          skip_drain, skip_dma, drain_eng, korder,
                          bench_opts)
        nc.compile()
        return nc

    descs = _mm_descs(mode)
    xins, wins = [], []
    for k, d in enumerate(descs):
        ki, pl = d["ki"], d["planes"]
        xins.append(nc.dram_tensor(f"x{k}", [ki, pl * B_PC * H * W],
                                   mybir.dt.float8e5, kind="ExternalInput").ap())
        wins.append(nc.dram_tensor(f"w{k}", [ki, pl * C_OUT],
                                   mybir.dt.float8e5, kind="ExternalInput").ap())
    bin_ = (nc.dram_tensor("bin", [C_OUT, 1], mybir.dt.float32,
                           kind="ExternalInput").ap() if has_bias else None)
    yout = nc.dram_tensor("yout", [C_OUT, B_PC * SPI], mybir.dt.float16,
                          kind="ExternalOutput").ap()

    with tile.TileContext(nc) as tc:
        with ExitStack() as ctx:
            _emit(tc, ctx, descs, xins, wins, yout, bin_, repeats, mode,
                  skip_drain, skip_dma, drain_eng, korder, bench_opts)
    nc.compile()
    return nc


def _emit_dq3(tc, ctx, xin, win, yout, bin_, repeats, skip_drain=False,
              skip_dma=False, drain_eng="alt", korder=False, bench_opts=None):
    bench_opts = bench_opts or {}
    nc = tc.nc
    f8, f16, f32 = mybir.dt.float8e5, mybir.dt.float16, mybir.dt.float32
    dr = mybir.MatmulPerfMode.DoubleRow

    singles = ctx.enter_context(tc.tile_pool(name="singles", bufs=1))
    psum_pool = ctx.enter_context(tc.tile_pool(name="ps", bufs=8, space="PSUM"))
    out_pool = ctx.enter_context(tc.tile_pool(name="outs", bufs=3))

    in_blocks = bench_opts.get("in_blocks", 4)
    warm = bench_opts.get("warm", 2)
    indma_per_rep = bench_opts.get("indma_per_rep", 0)
    rings = [nc.sync, nc.scalar]  # the two HWDGE rings (SP / Activation)

    xin3 = xin.rearrange("c (p s) -> c p s", p=2)
    win4 = win.rearrange("c (m p o) -> c m p o", m=3, p=2)

    if indma_per_rep:
        in_pool = ctx.enter_context(tc.tile_pool(name="ins", bufs=2))
        xq = wt = None
    else:
        xq = singles.tile([128, 2, LQ], f8, name="xq")
        wt = singles.tile([128, 3, 2, C_OUT], f8, name="wt")

    def load_inputs(xq, wt):
        # weights first (small) on SP so warmup matmuls can start early;
        # x in column blocks alternating rings so chunk-0 matmuls start
        # after the first block rather than the whole 1.7MB
        nc.sync.dma_start(wt[:], win4)
        bounds = np.linspace(0, LQ, in_blocks + 1).astype(int)
        for i in range(in_blocks):
            j0, j1 = int(bounds[i]), int(bounds[i + 1])
            rings[i % 2].dma_start(xq[:, :, j0:j1], xin3[:, :, j0:j1])

    if not indma_per_rep:
        load_inputs(xq, wt)

    bias_sb = None
    if bin_ is not None:
        bias_sb = singles.tile([C_OUT, 1], f32)
        nc.sync.dma_start(bias_sb[:], bin_[:])

    nmm = min(3, bench_opts.get("nmm_cap") or 3)
    same_w = bench_opts.get("same_w", False)

    def one_mm(pt, c, c0, fw, start, stop):
        wc = 0 if same_w else c
        nc.tensor.matmul(pt[:, :fw], wt[0:128, wc, :, :],
                         xq[0:128, :, c0 + c:c0 + c + fw],
                         start=start, stop=stop, perf_mode=dr,
                         skip_group_check=bool(korder))

    def drain_slice(t, ysl, psl):
        # only ACT and DVE can read PSUM (GPSIMD/Pool is verifier-rejected)
        if bias_sb is not None:
            nc.vector.tensor_scalar_add(ysl, psl, bias_sb[:, 0:1])
        elif drain_eng == "act" or (drain_eng == "alt" and t % 2 == 0):
            nc.scalar.copy(ysl, psl)
        else:
            nc.vector.tensor_copy(ysl, psl)

    dma_pg = bench_opts.get("dma_pg", DMA_PG)

    def emit_warm(xq_, wt_):
        # ramp the PE clock while the x blocks stream in: harmless f8
        # matmuls on the (small, already-resident) weight tile. PSUM comes
        # from the main pool tags so the 8-bank budget is respected.
        rhs = wt_[:].rearrange("k m p o -> k (m p o)")[:, 0:512]
        for wi in range(warm):
            if korder == "g4":
                pw = psum_pool.tile([C_OUT, 4, 512], f32, tag="ps4",
                                    bufs=2, name=f"warm{wi}")[:, 0, :]
            else:
                pw = psum_pool.tile([C_OUT, 512], f32, tag="ps", name=f"warm{wi}")
            nc.tensor.matmul(pw, wt_[0:128, 0, 0, :], rhs,
                             start=True, stop=True, skip_group_check=True)

    if not indma_per_rep and warm:
        emit_warm(xq, wt)

    for _rep in range(repeats):
        if indma_per_rep:
            xq = in_pool.tile([128, 2, LQ], f8, tag="xq")
            wt = in_pool.tile([128, 3, 2, C_OUT], f8, tag="wt")
            load_inputs(xq, wt)
            if warm:
                emit_warm(xq, wt)
        y16 = out_pool.tile([C_OUT, FREE_V], f16, tag="y16")
        if korder == "g4":
            for gi, g0 in enumerate(range(0, len(CHUNKS_P), 4)):
                grp = CHUNKS_P[g0:g0 + 4]
                pt = psum_pool.tile([C_OUT, 4, 512], f32, tag="ps4",
                                    bufs=2, name=f"pt{gi}")
                for c in range(nmm):
                    for b, (s0, d0, fw) in enumerate(grp):
                        one_mm(pt[:, b, :], c, s0, fw, c == 0, c == nmm - 1)
                if skip_drain:
                    continue
                gd0, gcw = grp[0][1], sum(fw for _, _, fw in grp)
                psl = pt[:, 0:len(grp), :].rearrange("o b f -> o (b f)")[:, :gcw]
                drain_slice(gi, y16[:, gd0:gd0 + gcw], psl)
                if dma_pg and not skip_dma:
                    rings[gi % 2].dma_start(yout[:, gd0:gd0 + gcw],
                                            y16[:, gd0:gd0 + gcw])
        elif korder == "ws":
            # weight-stationary sweeps over chunk groups (3 LD_WEIGHTS per
            # group); small last group so the rep tail is one drain + one
            # short DMA
            gb = [0, 8, 14] if bench_opts.get("gb2") else [0, 8, 13, 14]
            for gi in range(len(gb) - 1):
                grp = CHUNKS_P[gb[gi]:gb[gi + 1]]
                pts = [psum_pool.tile([C_OUT, 512], f32, tag="ps",
                                      name=f"pt{gb[gi] + i}")
                       for i in range(len(grp))]
                for c in range(nmm):
                    for pt, (s0, d0, fw) in zip(pts, grp):
                        one_mm(pt, c, s0, fw, c == 0, c == nmm - 1)
                if skip_drain:
                    continue
                for t, (pt, (s0, d0, fw)) in enumerate(zip(pts, grp)):
                    drain_slice(gb[gi] + t, y16[:, d0:d0 + fw], pt[:, :fw])
                if dma_pg and not skip_dma:
                    gd0 = grp[0][1]
                    gd1 = grp[-1][1] + grp[-1][2]
                    rings[gi % 2].dma_start(yout[:, gd0:gd1],
                                            y16[:, gd0:gd1])
        elif korder:
            # weight-stationary over 8-bank groups: 3 LD_WEIGHTS per group
            for g0 in range(0, len(CHUNKS_P), 8):
                grp = CHUNKS_P[g0:g0 + 8]
                pts = [psum_pool.tile([C_OUT, 512], f32, tag="ps",
                                      name=f"pt{g0 + i}")
                       for i in range(len(grp))]
                for c in range(nmm):
                    for pt, (s0, d0, fw) in zip(pts, grp):
                        one_mm(pt, c, s0, fw, c == 0, c == nmm - 1)
                if not skip_drain:
                    for t, (pt, (s0, d0, fw)) in enumerate(zip(pts, grp)):
                        drain_slice(g0 + t, y16[:, d0:d0 + fw], pt[:, :fw])
        else:
            # with dma_pg: issue the output DMA in groups as drains complete
            # (ring-alternated), keeping the LAST group small so the
            # post-compute tail is one drain + one short DMA
            nt = len(CHUNKS_P)
            bounds = ([] if not dma_pg else
                      [nt * (i + 1) // max(dma_pg - 1, 1)
                       for i in range(max(dma_pg - 1, 1))][:-1] + [nt - 1, nt])
            prev = 0
            for t, (s0, d0, fw) in enumerate(CHUNKS_P):
                pt = psum_pool.tile([C_OUT, 512], f32, tag="ps")
                for c in range(nmm):
                    one_mm(pt, c, s0, fw, c == 0, c == nmm - 1)
                if not skip_drain:
                    drain_slice(t, y16[:, d0:d0 + fw], pt[:, :fw])
                    if dma_pg and not skip_dma and t + 1 in bounds:
                        gd0 = CHUNKS_P[prev][1]
                        gd1 = d0 + fw
                        rings[len([b for b in bounds if b <= t]) % 2].dma_start(
                            yout[:, gd0:gd1], y16[:, gd0:gd1])
                        prev = t + 1
        if not (skip_drain or skip_dma or
                (dma_pg and korder in (False, "g4", "ws"))):
            nc.sync.dma_start(yout[:, :], y16[:])


def _emit(tc, ctx, descs, xins, wins, yout, bin_, repeats, mode,
          skip_drain=False, skip_dma=False, drain_eng="alt", korder=False,
          bench_opts=None):
    bench_opts = bench_opts or {}
    nc = tc.nc
    f8, f16, f32 = mybir.dt.float8e5, mybir.dt.float16, mybir.dt.float32
    dr = (mybir.MatmulPerfMode.DoubleRow if mode == "dr3" else None)

    singles = ctx.enter_context(tc.tile_pool(name="singles", bufs=1))
    psum_pool = ctx.enter_context(tc.tile_pool(name="ps", bufs=8, space="PSUM"))
    out_pool = ctx.enter_context(tc.tile_pool(name="outs", bufs=3))

    xgs, wts = [], []
    for k, d in enumerate(descs):
        ki, pl = d["ki"], d["planes"]
        xg = singles.tile([ki, pl, FREE], f8, name=f"xg{k}")
        wt = singles.tile([ki, pl, C_OUT], f8, name=f"wt{k}")
        nc.sync.dma_start(xg[0:ki], xins[k].rearrange(
            "c (p s) -> c p s", p=pl))
        nc.sync.dma_start(wt[0:ki], wins[k].rearrange(
            "c (p o) -> c p o", p=pl))
        xgs.append(xg)
        wts.append(wt)

    bias_sb = None
    if bin_ is not None:
        bias_sb = singles.tile([C_OUT, 1], f32)
        nc.sync.dma_start(bias_sb[:], bin_[:])

    nmm = len(descs)

    same_w = bench_opts.get("same_w", False)
    nmm_cap = bench_opts.get("nmm_cap")

    def one_mm(pt, k, c0, fw, start, stop):
        if same_w:
            k = 0
        d = descs[k]
        ki = d["ki"]
        if dr is not None:
            lhsT = wts[k][0:ki, :, :]
            rhs = xgs[k][0:ki, :, c0:c0 + fw]
        else:
            lhsT = wts[k][0:ki, 0, :]
            rhs = xgs[k][0:ki, 0, c0:c0 + fw]
        nc.tensor.matmul(pt[:, :fw], lhsT, rhs, start=start, stop=stop,
                         perf_mode=dr, skip_group_check=bool(korder))

    def drain(t, pt, c0, fw, y16):
        ysl = y16[:, c0:c0 + fw]
        if bias_sb is not None:
            nc.vector.tensor_scalar_add(ysl, pt[:, :fw], bias_sb[:, 0:1])
        elif drain_eng == "act" or (drain_eng == "alt" and t % 2 == 0):
            nc.scalar.copy(ysl, pt[:, :fw])
        else:
            nc.vector.tensor_copy(ysl, pt[:, :fw])

    for _rep in range(repeats):
        y16 = out_pool.tile([C_OUT, FREE], f16, tag="y16")
        if korder == "g4":
            # weight-stationary over 4-bank PSUM tiles; one fused drain
            # per group (4 banks contiguous in PSUM -> single ACT/DVE op)
            for gi, g0 in enumerate(range(0, len(CHUNKS), 4)):
                grp = CHUNKS[g0:g0 + 4]
                pt = psum_pool.tile([C_OUT, 4, 512], f32, tag="ps4",
                                    bufs=2, name=f"pt{gi}")
                for k in range(nmm):
                    for b, (c0, fw) in enumerate(grp):
                        one_mm(pt[:, b, :], k, c0, fw, k == 0, k == nmm - 1)
                if skip_drain:
                    continue
                gc0, gcw = grp[0][0], sum(fw for _, fw in grp)
                ysl = y16[:, gc0:gc0 + gcw]
                psl = pt[:, 0:len(grp), :].rearrange("o b f -> o (b f)") \
                    if gcw == 512 * len(grp) else pt[:, 0, :gcw]
                if bias_sb is not None:
                    nc.vector.tensor_scalar_add(ysl, psl, bias_sb[:, 0:1])
                elif drain_eng == "act" or (drain_eng == "alt" and gi % 2 == 0):
                    nc.scalar.copy(ysl, psl)
                else:
                    nc.vector.tensor_copy(ysl, psl)
        elif korder:
            # weight-stationary: for each PSUM-bank group, sweep each
            # stationary over all banks before switching weights
            for g0 in range(0, len(CHUNKS), 8):
                grp = CHUNKS[g0:g0 + 8]
                pts = [psum_pool.tile([C_OUT, 512], f32, tag="ps",
                                      name=f"pt{g0 + i}")
                       for i in range(len(grp))]
                for k in range(nmm):
                    for pt, (c0, fw) in zip(pts, grp):
                        one_mm(pt, k, c0, fw, k == 0, k == nmm - 1)
                if not skip_drain:
                    for t, (pt, (c0, fw)) in enumerate(zip(pts, grp)):
                        drain(g0 + t, pt, c0, fw, y16)
        else:
            for t, (c0, fw) in enumerate(CHUNKS):
                pt = psum_pool.tile([C_OUT, 512], f32, tag="ps")
                n_emit = min(nmm, nmm_cap) if nmm_cap else nmm
                for k in range(n_emit):
                    one_mm(pt, k, c0, fw, k == 0, k == n_emit - 1)
                if not skip_drain:
                    drain(t, pt, c0, fw, y16)
        if not (skip_drain or skip_dma):
            nc.sync.dma_start(yout[:, :], y16[:])


def _prep_inputs_dq3(x, weight):
    """dq3 layout: per core, 4 row-shifted copies of the padded flat plane
    at [partition group 0/1] x [plane 0/1]; shift r = g + 2p rows."""
    f8 = ml_dtypes.float8_e5m2
    xq = x.astype(f8)
    wq = weight.astype(f8)                       # [C_OUT, C_IN, K, K]

    wb = np.zeros((128, 3, 2, C_OUT), f8)
    for c in range(K):
        for g in range(2):
            for p in range(2):
                r = g + 2 * p
                if r < K:
                    wb[64 * g:64 * (g + 1), c, p, :] = wq[:, :, r, c].T
    wflat = np.ascontiguousarray(wb.reshape(128, 3 * 2 * C_OUT))

    in_maps = []
    for core in range(N_CORES):
        xs = xq[core * B_PC:(core + 1) * B_PC]   # [B_PC, C_IN, H, W]
        xp = np.zeros((B_PC, C_IN, HP, WP), f8)
        xp[:, :, PAD:PAD + H, PAD:PAD + W] = xs
        flat = xp.transpose(1, 0, 2, 3).reshape(C_IN, FREE_P)
        buf = np.zeros((128, 2, LQ), f8)
        for g in range(2):
            for p in range(2):
                r = g + 2 * p
                n = FREE_P - WP * r
                buf[64 * g:64 * (g + 1), p, :n] = flat[:, WP * r:]
        in_maps.append({"xin": np.ascontiguousarray(buf.reshape(128, 2 * LQ)),
                        "win": wflat})
    return in_maps


def _prep_inputs(x, weight, mode=MODE):
    """Host-side quantize + tap-pre-shifted layout. Per-core input maps."""
    if mode == "dq3":
        return _prep_inputs_dq3(x, weight)
    f8 = ml_dtypes.float8_e5m2
    descs = _mm_descs(mode)
    xq = x.astype(f8)
    wq = weight.astype(f8)                       # [C_OUT, C_IN, K, K]

    # weights: per mm, [Ki, planes*C_OUT]
    wbufs = []
    for d in descs:
        ki, pl = d["ki"], d["planes"]
        wb = np.zeros((ki, pl, C_OUT), f8)
        for p in range(pl):
            for g, tap in enumerate(d["taps"][p]):
                if tap is None:
                    continue
                ih, iw = tap
                if d.get("ch_split"):
                    cs = slice(p * ki, (p + 1) * ki)   # plane selects channels
                    wb[:, p, :] = wq[:, cs, ih, iw].T
                else:
                    wb[g * 64:(g + 1) * 64, p, :] = wq[:, :, ih, iw].T
        wbufs.append(np.ascontiguousarray(wb.reshape(ki, pl * C_OUT)))

    in_maps = []
    for core in range(N_CORES):
        xs = xq[core * B_PC:(core + 1) * B_PC]   # [B_PC, C_IN, H, W]
        xp = np.zeros((B_PC, C_IN, H + 2 * PAD, W + 2 * PAD), f8)
        xp[:, :, PAD:PAD + H, PAD:PAD + W] = xs
        m = {}
        for k, d in enumerate(descs):
            ki, pl = d["ki"], d["planes"]
            xb = np.zeros((ki, pl, B_PC, H, W), f8)
            for p in range(pl):
                for g, tap in enumerate(d["taps"][p]):
                    if tap is None:
                        continue
                    ih, iw = tap
                    win = xp[:, :, ih:ih + H, iw:iw + W]   # [B_PC, C_IN, H, W]
                    if d.get("ch_split"):
                        xb[:, p] = win[:, p * ki:(p + 1) * ki].transpose(1, 0, 2, 3)
                    else:
                        xb[g * 64:(g + 1) * 64, p] = win.transpose(1, 0, 2, 3)
            m[f"x{k}"] = np.ascontiguousarray(xb.reshape(ki, pl * B_PC * H * W))
            m[f"w{k}"] = wbufs[k]
        in_maps.append(m)
    return in_maps


def kernel(x, weight, bias, _trace=False):
    x = np.asarray(x, np.float32)
    weight = np.asarray(weight, np.float32)
    bias = np.asarray(bias, np.float32)
    has_bias = bool(np.any(bias))

    key = (MODE, KORDER, has_bias)
    if key not in _COMPILED:
        _COMPILED[key] = _build(has_bias=has_bias, korder=KORDER)
    nc = _COMPILED[key]

    in_maps = _prep_inputs(x, weight)
    if has_bias:
        # reference adds bias once per accumulation step (72 times total)
        beff = (NSTEP * bias).reshape(C_OUT, 1).astype(np.float32)
        for m in in_maps:
            m["bin"] = np.ascontiguousarray(beff)
    res = run_bass_kernel_spmd(nc, in_maps, list(range(N_CORES)),
                               trace=_trace)

    y = np.empty((B, C_OUT, H, W), np.float32)
    for core in range(N_CORES):
        yo = res.results[core]["yout"]
        if MODE == "dq3":
            yo = yo.reshape(C_OUT, B_PC, H, WP)[:, :, :, :W]
        else:
            yo = yo.reshape(C_OUT, B_PC, H, W)
        y[core * B_PC:(core + 1) * B_PC] = \
            yo.astype(np.float32).transpose(1, 0, 2, 3)
    if _trace:
        return y, res
    return y

